# revision 52
# baseline (speedup 1.0000x reference)
# Trainium2 Bass kernel for nn_AttentionPropagation (SuperGlue-style bidirectional
# attentional propagation): 6x (1x1conv+BN+ReLU) filters + QK attention with
# softmax over BOTH axes + two aggregations + output filters.
#
# Sharding: 16 (batch, head) units over 8 cores -> each core owns batch b=core//2
# and a contiguous 128-channel (2-head) slice of the filter outputs.  Core pairs
# {2b, 2b+1} AllGather normalized add0/add1 halves; output filters f4/f5 are
# split by OUTPUT channel across the pair.
#
# Single-exp dataflow (v2): the QK matrix is exponentiated ONCE (E-stream,
# n-major).  The m-major copy F = E^T needed for the add0 aggregation is made
# by 64 HWDGE xbar DMA-transposes ([128n,1024m] bf16 tile -> [m%128, mb, n]
# 3D view), which stream on the sync queue underneath the E-stream.  This
# deletes the entire second QK^T+exp stream (the old ACT bottleneck).
#   Softmax sums come from the PE, not ACT accumulators:
#     colsum (for add1): ones-column matmuls on E tiles (unpacked - they also
#       keep PE duty >100% of ACT so the HAM clock gate never drops)
#     rowsum (for add0): ones-column matmuls on F tiles, packed into spare
#       PE column-groups so they run concurrently with the U0 matmuls.
#   Normalizers: DVE fast-reciprocal of the sum rows + K=1 ones outer-product
#   broadcast on PE -> DVE multiply at eviction.
#   The E-stream is h2(m-half)-outer so U1 lives in a 2-bank PSUM slab; the
#   F-phase is nh(n-half)-outer so U0 does too.  Each half's add1/add0 chain +
#   pair-AllGather + af load + output filter overlaps the next half's compute.

import os

import numpy as np

DBG = os.environ.get("KDBG", "0") == "1"

B, N, M, C = 4, 2048, 2048, 256
H, Dh = 4, 64
EPS = 1e-5
NCORES = 8
LAG = 3

_CACHE = {}


def _build_program():
    from contextlib import ExitStack

    import concourse.bass as bass
    import concourse.tile as tile
    from concourse import bacc, mybir
    from concourse.bass import ts

    f32 = mybir.dt.float32
    bf16 = mybir.dt.bfloat16
    AF = mybir.ActivationFunctionType
    ALU = mybir.AluOpType

    nc = bacc.Bacc(
        "TRN2",
        target_bir_lowering=False,
        debug=False,
        enable_asserts=False,
        num_devices=NCORES,
    )

    # ---- DRAM I/O ----
    x1t_d = nc.dram_tensor("x1t", [128, 2 * N], bf16, kind="ExternalInput").ap()
    x2t_d = nc.dram_tensor("x2t", [128, 2 * M], bf16, kind="ExternalInput").ap()
    wq_d = nc.dram_tensor("wq", [128, 2 * 128], bf16, kind="ExternalInput").ap()
    wk_d = nc.dram_tensor("wk", [128, 2 * 128], bf16, kind="ExternalInput").ap()
    wv0_d = nc.dram_tensor("wv0", [128, 2 * 128], bf16, kind="ExternalInput").ap()
    wv1_d = nc.dram_tensor("wv1", [128, 2 * 128], bf16, kind="ExternalInput").ap()
    bq_d = nc.dram_tensor("bq", [128, 1], f32, kind="ExternalInput").ap()
    bk_d = nc.dram_tensor("bk", [128, 1], f32, kind="ExternalInput").ap()
    bv0_d = nc.dram_tensor("bv0", [128, 1], f32, kind="ExternalInput").ap()
    bv1_d = nc.dram_tensor("bv1", [128, 1], f32, kind="ExternalInput").ap()
    bv1r_d = nc.dram_tensor("bv1r", [1, 128], bf16, kind="ExternalInput").ap()
    w4h_d = nc.dram_tensor("w4h", [128, 2 * 128], bf16, kind="ExternalInput").ap()
    w5h_d = nc.dram_tensor("w5h", [128, 2 * 128], bf16, kind="ExternalInput").ap()
    b4h_d = nc.dram_tensor("b4h", [1, 128], bf16, kind="ExternalInput").ap()
    b5h_d = nc.dram_tensor("b5h", [1, 128], bf16, kind="ExternalInput").ap()
    ones_d = nc.dram_tensor("ones", [1, 128], bf16, kind="ExternalInput").ap()
    onesp_d = nc.dram_tensor("onesp", [128, 1], bf16, kind="ExternalInput").ap()
    onesb_d = nc.dram_tensor("onesb", [128, 128], bf16, kind="ExternalInput").ap()
    sc2_d = nc.dram_tensor("sc2", [32, 128], bf16, kind="Internal").ap()
    identb_d = nc.dram_tensor("identb", [128, 128], bf16, kind="ExternalInput").ap()
    out0_d = nc.dram_tensor("out0h", [128, 16 * 128], f32, kind="ExternalOutput").ap()
    out1_d = nc.dram_tensor("out1h", [128, 16 * 128], f32, kind="ExternalOutput").ap()
    SEGW = [1024, 512, 512]
    SEGO = [0, 1024, 1536]
    ccb_in = nc.dram_tensor("ccb_in", [256, M], bf16, kind="Internal").ap()
    ccb_out = nc.dram_tensor("ccb_out", [512, M], bf16, kind="Internal").ap()
    ccp_in = nc.dram_tensor("ccp_in", [128, 16], bf16, kind="Internal").ap()
    ccp_out = nc.dram_tensor("ccp_out", [256, 16], bf16, kind="Internal").ap()
    if DBG:
        dbg = {
            "d_q": nc.dram_tensor("d_q", [128, N], bf16, kind="ExternalOutput").ap(),
            "d_k": nc.dram_tensor("d_k", [128, M], bf16, kind="ExternalOutput").ap(),
            "d_v1t": nc.dram_tensor("d_v1t", [128, 2048], bf16, kind="ExternalOutput").ap(),
            "d_et": nc.dram_tensor("d_et", [128, 1024], bf16, kind="ExternalOutput").ap(),
            "d_f0": nc.dram_tensor("d_f0", [128, 2048], bf16, kind="ExternalOutput").ap(),
            "d_u1sb": nc.dram_tensor("d_u1sb", [128, M], bf16, kind="ExternalOutput").ap(),
            "d_add1": nc.dram_tensor("d_add1", [128, M], bf16, kind="ExternalOutput").ap(),
            "d_add0": nc.dram_tensor("d_add0", [128, N], bf16, kind="ExternalOutput").ap(),
            "d_rec1": nc.dram_tensor("d_rec1", [33, 2048], f32, kind="ExternalOutput").ap(),
            "d_rec0": nc.dram_tensor("d_rec0", [33, 2048], f32, kind="ExternalOutput").ap(),
            "d_af1": nc.dram_tensor("d_af1", [128, 2 * M], bf16, kind="ExternalOutput").ap(),
            "d_af0": nc.dram_tensor("d_af0", [128, 2 * N], bf16, kind="ExternalOutput").ap(),
        }

    NB = N // 128  # 16 n-blocks
    MB = M // 128  # 16 m-blocks
    PAIRS = [[0, 1], [2, 3], [4, 5], [6, 7]]

    with tile.TileContext(nc) as tc, ExitStack() as ctx:
        const = ctx.enter_context(tc.tile_pool(name="const", bufs=1))
        # x1t/x2t (phase 1) and af0/af1 (phase 4+) share two slots via one tag
        xpool = ctx.enter_context(tc.tile_pool(name="xp", bufs=2))
        qkp = ctx.enter_context(tc.tile_pool(name="qkp", bufs=1))
        vp = ctx.enter_context(tc.tile_pool(name="vp", bufs=1))
        fp = ctx.enter_context(tc.tile_pool(name="fp", bufs=1))
        epool = ctx.enter_context(tc.tile_pool(name="ep", bufs=7))
        sbp = ctx.enter_context(tc.tile_pool(name="sbp", bufs=1))
        rbp = ctx.enter_context(tc.tile_pool(name="rbp", bufs=1))
        recp = ctx.enter_context(tc.tile_pool(name="recp", bufs=2))
        opool = ctx.enter_context(tc.tile_pool(name="opool", bufs=2))
        # PSUM: pA = 2x[128,1024] (QK rotation / U0 halves / filters / psO)
        #       pB = 1x[128,1024] (U1 half / rowsum acc)
        #       pC = 1x[128,1024] (colsum acc / f5 psums / psO-f)
        pA = ctx.enter_context(tc.tile_pool(name="pA", bufs=2, space="PSUM"))
        pB = ctx.enter_context(tc.tile_pool(name="pB", bufs=1, space="PSUM"))
        pC = ctx.enter_context(tc.tile_pool(name="pC", bufs=1, space="PSUM"))

        # ---- inputs: x2t first (k filter unblocks the E-stream) ----
        x1t_sb = xpool.tile([128, 2, N], bf16, tag="xa")
        x2t_sb = xpool.tile([128, 2, M], bf16, tag="xa")
        x2t_v = x2t_d.rearrange("p (a n) -> p a n", a=2)
        x1t_v = x1t_d.rearrange("p (a n) -> p a n", a=2)
        nc.sync.dma_start(x2t_sb[:, 0], x2t_v[:, 0])
        nc.sync.dma_start(x2t_sb[:, 1], x2t_v[:, 1])

        wq_sb = const.tile([128, 2, 128], bf16, tag="wq")
        wk_sb = const.tile([128, 2, 128], bf16, tag="wk")
        wv0_sb = const.tile([128, 2, 128], bf16, tag="wv0")
        wv1_sb = const.tile([128, 2, 128], bf16, tag="wv1")
        w4h_sb = const.tile([128, 2, 128], bf16, tag="w4h")
        w5h_sb = const.tile([128, 2, 128], bf16, tag="w5h")
        bq_sb = const.tile([128, 1], f32, tag="bq")
        bk_sb = const.tile([128, 1], f32, tag="bk")
        bv0_sb = const.tile([128, 1], f32, tag="bv0")
        bv1_sb = const.tile([128, 1], f32, tag="bv1")
        b4h_sb = const.tile([1, 128], bf16, tag="b4h")
        b5h_sb = const.tile([1, 128], bf16, tag="b5h")
        ones_t = const.tile([1, 128], bf16, tag="ones")
        onesp_sb = const.tile([128, 1], bf16, tag="onesp")
        onesb_sb = const.tile([128, 128], bf16, tag="onesb")
        bv1r_sb = const.tile([1, 128], bf16, tag="bv1r")
        identb_sb = const.tile([128, 128], bf16, tag="identb")
        nc.scalar.dma_start(identb_sb[:], identb_d)
        nc.scalar.dma_start(wk_sb[:], wk_d.rearrange("p (a d) -> p a d", a=2))
        nc.scalar.dma_start(bk_sb[:], bk_d)
        nc.scalar.dma_start(x1t_sb[:, 0], x1t_v[:, 0])
        nc.scalar.dma_start(x1t_sb[:, 1], x1t_v[:, 1])
        for dst, src in (
            (wq_sb, wq_d), (wv1_sb, wv1_d), (wv0_sb, wv0_d),
        ):
            nc.scalar.dma_start(dst[:], src.rearrange("p (a d) -> p a d", a=2))
        for dst, src in (
            (bq_sb, bq_d), (bv1_sb, bv1_d), (bv1r_sb, bv1r_d), (bv0_sb, bv0_d),
            (ones_t, ones_d), (onesp_sb, onesp_d), (onesb_sb, onesb_d),
            (b4h_sb, b4h_d), (b5h_sb, b5h_d),
        ):
            nc.scalar.dma_start(dst[:], src)
        for dst, src in ((w4h_sb, w4h_d), (w5h_sb, w5h_d)):
            nc.scalar.dma_start(dst[:], src.rearrange("p (a d) -> p a d", a=2))

        # HAM warm-up: full-contraction accumulating matmuls through a full
        # 4096-cycle window, flipping the clock gate to 8/8 before phase 1.
        bps = pA.tile([128, 1024], f32, tag="s")
        for i in range(16):
            nc.tensor.matmul(
                bps[:, 0:512], identb_sb[:], x2t_sb[:, 0, 0:512],
                start=(i == 0), stop=(i == 15),
            )

        # ---- phase 1: q/k filter chunks (ACT relu+bias eviction) interleaved
        # with v1t/v0t 4-block rotations (DVE eviction)
        q_sb = qkp.tile([128, N], bf16, tag="q")
        k_sb = qkp.tile([128, M], bf16, tag="k")
        v0t_sb = vp.tile([128, MB, 128], bf16, tag="v0t")  # [m-in-block, mb, d]
        v1t_sb = vp.tile([128, NB * 128], bf16, tag="v1t")
        # v0 computed in [d, m] layout (dense 512-wide chunks) then
        # xbar-DMA-transposed at E-END into [m-in-block, mb, d]; v1t is built
        # directly by out-transposed PE matmuls (v_rot) because a sync-ring
        # transpose here would queue behind the dummy collective's wind-down
        # and stall the first U1 matmuls ~10us.
        v0dk = sbp.tile([128, M], bf16, tag="a0", name="v0dk")
        # F = E^T storage, [m-in-block, mb, n] per head
        f_sb = [
            fp.tile([128, MB, N], bf16, tag=f"f{u}", name=f"f{u}") for u in range(2)
        ]

        def qk_chunk(dst, xt, w, bias, j, on_act):
            ps = pA.tile([128, 1024], f32, tag="s")
            p5 = ps[:, 0:512]
            nc.tensor.matmul(
                p5, w[:, 0], xt[:, 0, ts(j, 512)], start=True, stop=False,
            )
            nc.tensor.matmul(
                p5, w[:, 1], xt[:, 1, ts(j, 512)], start=False, stop=True,
            )
            if on_act:
                nc.scalar.activation(dst[:, ts(j, 512)], p5, AF.Relu, bias=bias[:])
            else:
                nc.vector.tensor_scalar(
                    dst[:, ts(j, 512)], p5, bias[:], 0.0, op0=ALU.add, op1=ALU.max
                )

        def v_rot(dst, xt, w, brow, g):
            ps = pA.tile([128, 1024], f32, tag="s")
            p4 = ps[:, 0:512]
            for blk in range(4):
                mb = 4 * g + blk
                sub = p4[:, ts(blk, 128)]
                nc.tensor.matmul(
                    sub, xt[:, 0, ts(mb, 128)], w[:, 0], start=True, stop=False,
                )
                nc.tensor.matmul(
                    sub, xt[:, 1, ts(mb, 128)], w[:, 1], start=False, stop=False,
                )
                nc.tensor.matmul(
                    sub, ones_t[:, 0:128], brow[:], start=False, stop=True,
                )
            nc.vector.tensor_scalar_max(dst[:, ts(g, 512)], p4, 0.0)

        for j in range(4):
            qk_chunk(k_sb, x2t_sb, wk_sb, bk_sb, j, on_act=False)
            v_rot(v1t_sb, x1t_sb, wv1_sb, bv1r_sb, j)
        p1w = pA.tile([128, 1024], f32, tag="s")
        for i in range(6):
            nc.tensor.matmul(
                p1w[:, 0:512], identb_sb[:], x2t_sb[:, 1, 0:512],
                start=(i == 0), stop=(i == 5),
            )
        for j in range(4):
            qk_chunk(q_sb, x1t_sb, wq_sb, bq_sb, j, on_act=True)
            qk_chunk(v0dk, x2t_sb, wv0_sb, bv0_sb, j, on_act=False)
        # v0t is only consumed by the F phase: its transpose is emitted at the
        # END of the E-stream so it never gates the E transposes on the sync
        # ring (it previously delayed the first E transpose by ~25us).

        # staging for unnormalized U1 / normalized adds
        u1sb = sbp.tile([128, M], bf16, tag="u1sb", name="u1sb")
        add1_sb = sbp.tile([128, M], bf16, tag="a1", name="add1")
        add0_sb = sbp.tile([128, N], bf16, tag="a0", name="add0")
        af1 = xpool.tile([128, 2, M], bf16, tag="xa")  # reuses x1t slot
        af0 = xpool.tile([128, 2, N], bf16, tag="xa")  # reuses x2t slot

        # ---- E-stream: per m-half h2: QK -> exp -> (lagged) U1 + colsum
        # matmuls; every exp tile is also xbar-DMA-transposed into f_sb.
        # PE work per tile (QK 1024 + U1 1024 + colsum 1024 cyc) slightly
        # exceeds ACT (exp ~1.17us), so the PE stays dense and HAM-warm.
        def emit_u1cs(u, nb, et, U1h, csum):
            # colsum ones-matmuls are col-group-PACKED against U1: u=0's U1
            # occupies array col groups 0-1 so its colsum row sits at 96
            # (group 3); u=1 (groups 2-3) puts it at 32 (group 1).  Emitted
            # interleaved so each cs matmul runs concurrently with a U1 one.
            crow = 96 if u == 0 else 32
            for j in range(2):
                nc.tensor.matmul(
                    U1h[64 * u : 64 * u + 64, ts(j, 512)],
                    v1t_sb[:, nb * 128 + 64 * u : nb * 128 + 64 * u + 64],
                    et[:, ts(j, 512)],
                    start=(nb == 0), stop=(nb == NB - 1),
                    tile_position=(0, 64 * u),
                )
                nc.tensor.matmul(
                    csum[crow : crow + 1, ts(j, 512)],
                    onesp_sb[:],
                    et[:, ts(j, 512)],
                    start=(nb == 0), stop=(nb == NB - 1),
                    tile_position=(0, crow),
                )

        # deferred normalizer chains: PE outer-product parts are emitted a few
        # tiles into the NEXT phase so the strict-FIFO PE queue never stalls
        # waiting on the DVE reciprocal.  rec tiles: [2,1024] (row u = head u's
        # 1/sum), one rotating tag shared across all four chains.
        def chain1_pe(h2, sumrow):
            # broadcast the RAW colsum rows (K=1 ones outer-product) -> psO,
            # then reciprocal_approx_fast on the full base-0 [128,1024] tile
            # (the custom DVE op silently corrupts at partition base != 0).
            # Both chains fire inside the F phase (collectives must never
            # overlap the E-stream's DMA transposes - they freeze them), so
            # psO comes from pB, free once both U1 halves are evicted; pA's
            # two slots must stay reserved for the two U0 halves.
            psO = pB.tile([128, 1024], f32, tag="u1")
            for u in range(2):
                row = 96 if u == 0 else 32
                for j in range(2):
                    nc.tensor.matmul(
                        psO[64 * u : 64 * u + 64, ts(j, 512)],
                        onesb_sb[row : row + 1, 0:64],
                        sumrow[row : row + 1, ts(j, 512)],
                        start=True, stop=True,
                        tile_position=(row, 64 * u),
                    )
            rbc = rbp.tile([128, 1024], f32, tag="rbc")
            nc.vector.reciprocal_approx_fast(rbc[:], psO[:])
            half = slice(h2 * 1024, (h2 + 1) * 1024)
            nc.vector.tensor_mul(add1_sb[:, half], u1sb[:, half], rbc[:])
            if h2 == 1:
                # stage add1 into the merged gather buffer early; the single
                # AllGather fires once add0 lands too (chain0 seg 2)
                nc.gpsimd.dma_start(ccb_in[0:128, :], add1_sb[:])

        pend_chain = []  # (countdown, fn, phase)

        def tick_chains(phase):
            for item in list(pend_chain):
                if item[2] != phase:
                    continue
                item[0] -= 1
                if item[0] <= 0:
                    item[1]()
                    pend_chain.remove(item)

        # per-h2 rowsum accumulators (ACT accum_out), col = u*16+nb
        rows_acc = [
            sbp.tile([128, 32], f32, tag=f"ra{h2}", name=f"rows_acc{h2}")
            for h2 in range(2)
        ]

        def burst(ps, n, rhs):
            # full-array HAM re-warm burst: the half-array stream matmuls do
            # not register enough PE activity to hold the clock gate at 8/8,
            # so periodically hammer all 128x128 cells with real data.
            for i in range(n):
                nc.tensor.matmul(
                    ps[:, 0:512], identb_sb[:], rhs,
                    start=(i == 0), stop=(i == n - 1),
                )

        for h2 in range(2):
            U1h = pB.tile([128, 1024], f32, tag="u1")
            csum = pC.tile([128, 1024], f32, tag="cs")
            pend = []
            for nb in range(NB):
                for u in range(2):
                    t_idx = h2 * 32 + nb * 2 + u
                    ps = pA.tile([128, 1024], f32, tag="s")
                    if t_idx < 2:
                        burst(ps, 16, x2t_sb[:, 0, 0:512])
                    elif (t_idx % 4 == 2 and t_idx < 18) or t_idx % 8 == 4:
                        burst(ps, 6, x2t_sb[:, 1, ts(nb, 512) if nb < 4 else ts(0, 512)])
                    qs = q_sb[64 * u : 64 * u + 64, ts(nb, 128)]
                    for j in range(2):
                        nc.tensor.matmul(
                            ps[:, ts(j, 512)], qs,
                            k_sb[64 * u : 64 * u + 64,
                                 h2 * 1024 + 512 * j : h2 * 1024 + 512 * (j + 1)],
                            start=True, stop=True,
                        )
                    et = epool.tile([128, 1024], bf16, tag="et")
                    nc.scalar.activation(
                        et[:], ps[:], AF.Exp, scale=0.125,
                        accum_out=rows_acc[h2][:, u * 16 + nb : u * 16 + nb + 1],
                    )
                    if DBG and h2 == 0 and nb == 0 and u == 0:
                        nc.gpsimd.dma_start(dbg["d_et"], et[:])
                    nc.sync.dma_start(
                        f_sb[u][:, h2 * 8 : (h2 + 1) * 8, ts(nb, 128)],
                        et[:],
                        transpose=True,
                    )
                    pend.append((u, nb, et))
                    if len(pend) > LAG:
                        emit_u1cs(*pend.pop(0), U1h, csum)
                    tick_chains("E")
            for item in pend:
                emit_u1cs(*item, U1h, csum)
            # half-end (DVE parts now; PE outer-product deferred ~6 tiles so
            # the strict-FIFO PE queue never waits on the DVE reciprocals)
            half = slice(h2 * 1024, (h2 + 1) * 1024)
            nc.vector.tensor_copy(u1sb[:, half], U1h[:])
            sumrow = recp.tile([97, 1024], bf16, tag="rec", name=f"sum1_{h2}")
            nc.vector.tensor_copy(sumrow[:], csum[0:97, :])
            if DBG:
                nc.gpsimd.dma_start(
                    dbg["d_rec1"][:, h2 * 1024 : (h2 + 1) * 1024], sumrow[0:33, :]
                )
            pend_chain.append(
                [2 + 4 * h2, (lambda hh, rr: lambda: chain1_pe(hh, rr))(h2, sumrow),
                 "F"]
            )

        # ---- rowsum normalizer prep (runs during early F): combine the two
        # h2 accumulators, reciprocal (base-0 [128,32] - legal), then the
        # cross-partition reorder through DRAM into per-u flat rows.
        nc.sync.dma_start(v0t_sb[:], v0dk[:], transpose=True)
        rows16 = sbp.tile([128, 32], f32, tag="r16", name="rows16")
        nc.vector.tensor_add(rows16[:], rows_acc[0][:], rows_acc[1][:])
        rec32 = sbp.tile([128, 32], f32, tag="rc32", name="rec32")
        nc.vector.reciprocal_approx_fast(rec32[:], rows16[:])
        # cross-partition flatten via one xbar transpose + contiguous DRAM
        # round-trip (an element-strided gather here costs ~33us of SWDGE
        # descriptor time and stalls every collective queued behind it)
        rec32p = sbp.tile([128, 128], bf16, tag="rc32b", name="rec32p")
        nc.vector.memset(rec32p[:], 0.0)
        nc.vector.tensor_copy(rec32p[:, 0:32], rec32[:])
        recT = sbp.tile([128, 128], bf16, tag="rcT", name="recT")
        nc.sync.dma_start(recT[:], rec32p[:], transpose=True)
        nc.gpsimd.dma_start(sc2_d, recT[0:32, :])
        # sumflat rows: 0 = 1/rowsum head u0, 32 = head u1; cols n = nb*128+p
        sumflat = sbp.tile([33, 2048], bf16, tag="sflat", name="sumflat")
        sc_v = sc2_d.rearrange("(u x) p -> u (x p)", u=2)
        nc.gpsimd.dma_start(sumflat[0:1, :], sc_v[0:1])
        nc.gpsimd.dma_start(sumflat[32:33, :], sc_v[1:2])

        # ---- F-phase: n-segments [1024, 512, 512]: U0 from f_sb tiles.
        # Rowsums already in hand (ACT accum_out), so pure U0 matmuls.  The
        # two final quarters shrink the last AllGather on the tail.
        U0hs = [None, None, None]

        def chain0_pe(seg, _unused=None):
            # psO from pC: taking it from pA would deadlock (its slot is the
            # U0 bank whose eviction is this very chain's tensor_mul).
            # sumflat rows already hold 1/rowsum, so no reciprocal here.
            off, w = SEGO[seg], SEGW[seg]
            psO = pC.tile([128, 1024], f32, tag="cs")
            for u in range(2):
                row = 32 * u
                for j in range(w // 512):
                    nc.tensor.matmul(
                        psO[64 * u : 64 * u + 64, ts(j, 512)],
                        onesb_sb[row : row + 1, 0:64],
                        sumflat[row : row + 1, off + 512 * j : off + 512 * (j + 1)],
                        start=True, stop=True,
                        tile_position=(row, 64 * u),
                    )
            rbc = rbp.tile([128, 1024], f32, tag="rbc")
            nc.vector.tensor_copy(rbc[:, 0:w], psO[:, 0:w])
            half = slice(off, off + w)
            # eviction fused with normalize: add0 seg = U0 * (1/rowsum)
            nc.vector.tensor_mul(add0_sb[:, half], U0hs[seg][:, 0:w], rbc[:, 0:w])
            if seg == 2:
                # ONE rendezvous for both directions: with partner skew
                # dominant, a single 2MB gather beats two serialized ones
                nc.gpsimd.dma_start(ccb_in[128:256, :], add0_sb[:])
                nc.gpsimd.collective_compute(
                    "AllGather", ALU.bypass, replica_groups=PAIRS,
                    ins=[ccb_in], outs=[ccb_out],
                )
                nc.sync.dma_start(af1[:, 0, :], ccb_out[0:128, :])
                nc.sync.dma_start(af1[:, 1, :], ccb_out[256:384, :])
                nc.sync.dma_start(af0[:, 0, :], ccb_out[128:256, :])
                nc.sync.dma_start(af0[:, 1, :], ccb_out[384:512, :])

        def filt_g(out_d, af, wt, brow, g, pool, eng):
            ps = pool.tile([128, 1024], f32, tag="s" if pool is pA else "cs",
                           name=f"fg{g}")
            p4 = ps[:, 0:512]
            for blk in range(4):
                nb = 4 * g + blk
                sub = p4[:, ts(blk, 128)]
                nc.tensor.matmul(
                    sub, af[:, 0, ts(nb, 128)], wt[:, 0], start=True, stop=False,
                )
                nc.tensor.matmul(
                    sub, af[:, 1, ts(nb, 128)], wt[:, 1], start=False, stop=False,
                )
                nc.tensor.matmul(
                    sub, ones_t[:, 0:128], brow[:], start=False, stop=True,
                )
            ot = opool.tile([128, 512], f32, tag="ot")
            nc.vector.tensor_scalar_max(ot[:], p4, 0.0)
            eng.dma_start(out_d[:, ts(g, 512)], ot[:])

        # output filter half (f4 on af0 / f5 on af1), out cols [2*nh, 2*nh+1]
        def filt_half(out_d, af, wt, brow, nh, pool, eng):
            for g in (2 * nh, 2 * nh + 1):
                ps = pool.tile([128, 1024], f32, tag="s" if pool is pA else "cs")
                p4 = ps[:, 0:512]
                for blk in range(4):
                    nb = 4 * g + blk
                    sub = p4[:, ts(blk, 128)]
                    nc.tensor.matmul(
                        sub, af[:, 0, ts(nb, 128)], wt[:, 0], start=True, stop=False,
                    )
                    nc.tensor.matmul(
                        sub, af[:, 1, ts(nb, 128)], wt[:, 1], start=False, stop=False,
                    )
                    nc.tensor.matmul(
                        sub, ones_t[:, 0:128], brow[:], start=False, stop=True,
                    )
                ot = opool.tile([128, 512], f32, tag="ot")
                nc.vector.tensor_scalar_max(ot[:], p4, 0.0)
                eng.dma_start(out_d[:, ts(g, 512)], ot[:])

        fwarm = pA.tile([128, 1024], f32, tag="s")
        burst(fwarm, 10, f_sb[0][:, 0, 0:512])
        for seg in range(3):
            off, w = SEGO[seg], SEGW[seg]
            U0h = pA.tile([128, 1024], f32, tag="s")
            U0hs[seg] = U0h
            for mb in range(MB):
                if seg == 0 and mb % 6 == 3:
                    fw = pC.tile([128, 1024], f32, tag="cs", name=f"fw{seg}{mb}")
                    burst(fw, 6, f_sb[0][:, mb, 0:512])
                for u in range(2):
                    vsl = v0t_sb[:, mb, 64 * u : 64 * u + 64]
                    for j in range(w // 512):
                        fsl = f_sb[u][:, mb, off + 512 * j : off + 512 * (j + 1)]
                        nc.tensor.matmul(
                            U0h[64 * u : 64 * u + 64, ts(j, 512)],
                            vsl, fsl,
                            start=(mb == 0), stop=(mb == MB - 1),
                            tile_position=(0, 64 * u),
                        )
                tick_chains("F")
            if DBG and seg == 0:
                nc.gpsimd.dma_start(dbg["d_rec0"], sumflat[:])
            pend_chain.append(
                [2, (lambda hh: lambda: chain0_pe(hh))(seg), "F"]
            )

        # fire the last segment's chain NOW so its AllGather overlaps the
        # f5/f4 filter matmuls below
        tick_chains("F")
        tick_chains("F")

        # tail: f5 halves (af1 gathered during F), f4 g0+g1 (af0 seg0
        # gathered during seg1/seg2), flush the last chain, then f4 g2
        # (overlaps the final AllGather) and f4 g3 after it
        filt_half(out1_d, af1, w5h_sb, b5h_sb, 0, pC, nc.scalar)
        filt_half(out1_d, af1, w5h_sb, b5h_sb, 1, pC, nc.scalar)
        # only ONE pA tile may be taken before the flush (the other pA slot
        # still holds the un-evicted U0 of the last segment)
        filt_g(out0_d, af0, w4h_sb, b4h_sb, 0, pA, nc.sync)
        while pend_chain:
            tick_chains("F")
        for g in (1, 2, 3):
            filt_g(out0_d, af0, w4h_sb, b4h_sb, g, pA, nc.sync)
        if DBG:
            nc.gpsimd.dma_start(dbg["d_q"], q_sb[:])
            nc.gpsimd.dma_start(dbg["d_k"], k_sb[:])
            nc.gpsimd.dma_start(dbg["d_v1t"], v1t_sb[:])
            nc.gpsimd.dma_start(dbg["d_f0"], f_sb[0][:, 0, :])
            nc.gpsimd.dma_start(dbg["d_u1sb"], u1sb[:])
            nc.gpsimd.dma_start(dbg["d_add1"], add1_sb[:])
            nc.gpsimd.dma_start(dbg["d_add0"], add0_sb[:])
            nc.gpsimd.dma_start(
                dbg["d_af1"], af1[:].rearrange("p a n -> p (a n)")
            )
            nc.gpsimd.dma_start(
                dbg["d_af0"], af0[:].rearrange("p a n -> p (a n)")
            )

    nc.compile()
    return nc


def _prep_core_inputs(inputs):
    """Fold BN into weights, build per-core input maps."""
    x1 = np.ascontiguousarray(inputs["x1"], dtype=np.float32)
    x2 = np.ascontiguousarray(inputs["x2"], dtype=np.float32)
    Ws = np.asarray(inputs["Ws"], dtype=np.float32)
    bs = np.asarray(inputs["bs"], dtype=np.float32)
    g = np.asarray(inputs["gammas"], dtype=np.float32)
    be = np.asarray(inputs["betas"], dtype=np.float32)
    mn = np.asarray(inputs["means"], dtype=np.float32)
    vr = np.asarray(inputs["vars_"], dtype=np.float32)

    s = g / np.sqrt(vr + EPS)  # [6, C]
    Wf = Ws * s[:, :, None]  # rows scaled
    bf = s * (bs - mn) + be

    import ml_dtypes

    bfl = ml_dtypes.bfloat16

    def fold128(a):  # [C, X] -> [128, 2*X] partition-contiguous
        X = a.shape[1]
        return np.ascontiguousarray(
            a.reshape(2, 128, X).transpose(1, 0, 2).reshape(128, 2 * X)
        )

    WfT = np.ascontiguousarray(np.swapaxes(Wf, 1, 2)).astype(bfl)  # [6, C, C]
    x1t = np.ascontiguousarray(np.swapaxes(x1, 1, 2)).astype(bfl)  # [B, C, N]
    x2t = np.ascontiguousarray(np.swapaxes(x2, 1, 2)).astype(bfl)
    bfb = bf.astype(bfl)
    ident = np.eye(128, dtype=np.float32)

    in_maps = []
    for core in range(NCORES):
        b, par = core // 2, core % 2
        sl = slice(par * 128, par * 128 + 128)
        in_maps.append(
            {
                "x1t": fold128(x1t[b]),
                "x2t": fold128(x2t[b]),
                "wq": fold128(WfT[0][:, sl]),
                "wk": fold128(WfT[1][:, sl]),
                "wv0": fold128(WfT[2][:, sl]),
                "wv1": fold128(WfT[3][:, sl]),
                "bq": np.ascontiguousarray(bf[0][sl]).reshape(128, 1),
                "bk": np.ascontiguousarray(bf[1][sl]).reshape(128, 1),
                "bv0": np.ascontiguousarray(bf[2][sl]).reshape(128, 1),
                "bv1": np.ascontiguousarray(bf[3][sl]).reshape(128, 1),
                "bv1r": np.ascontiguousarray(bfb[3][sl]).reshape(1, 128),
                "w4h": fold128(WfT[4][:, sl]),
                "w5h": fold128(WfT[5][:, sl]),
                "b4h": np.ascontiguousarray(bfb[4][sl]).reshape(1, 128),
                "b5h": np.ascontiguousarray(bfb[5][sl]).reshape(1, 128),
                "ones": np.ones((1, 128), bfl),
                "onesp": np.ones((128, 1), bfl),
                "onesb": np.ones((128, 128), bfl),
                "identb": ident.astype(bfl),
            }
        )
    return in_maps


def _gather_outputs(results):
    """results[core][out{0,1}h] is [128, 16*128] partition-major; un-transpose
    to [2048, 128] and concat the pair's och halves."""

    def unfold(a):  # [128, 2048] -> [2048(n), 128(och)]
        return (
            np.ascontiguousarray(
                a.reshape(128, 16, 128).transpose(1, 0, 2)
            ).reshape(2048, 128)
        )

    out0 = np.stack(
        [
            np.concatenate(
                [unfold(results[2 * b]["out0h"]), unfold(results[2 * b + 1]["out0h"])],
                axis=1,
            )
            for b in range(B)
        ]
    )
    out1 = np.stack(
        [
            np.concatenate(
                [unfold(results[2 * b]["out1h"]), unfold(results[2 * b + 1]["out1h"])],
                axis=1,
            )
            for b in range(B)
        ]
    )
    return out0, out1


def kernel(**inputs):
    from concourse import bass_utils

    if "nc" not in _CACHE:
        _CACHE["nc"] = _build_program()
    nc = _CACHE["nc"]

    in_maps = _prep_core_inputs(inputs)
    res = bass_utils.run_bass_kernel_spmd(
        nc, in_maps, core_ids=list(range(NCORES))
    )
    return _gather_outputs(res.results)
